# revision 1
# baseline (speedup 1.0000x reference)
"""Trainium2 Bass kernel for causal top-K GNN message passing.

reference semantics (B=4, T=2048, D=1024, K=8):
    scores = x @ x^T per batch, causal (j <= i)
    A[i,j] = 1 iff j among top-8 causal scores of row i
    msg    = (A @ x) / deg
    out    = gelu(mix*x + (1-mix)*msg) * scale       (gain=*, bias=+ applied generally)

Strategy (8 NeuronCores, SPMD single program):
  - core c handles batch b = c % 4; cores 0-3 take row-tiles t = 15-2g
    (slot g = 0..7), cores 4-7 take t = 14-2g.
  - slot g is compiled for causal width W_g = 128*(16-2g) columns; cores 4-7
    use a per-core pair-swapped row-block permutation of the key/value axis so
    their row-tile lands in the last 128 columns of the slot's width. All
    per-core variation lives in the host-prepared input data; the device
    program is identical across cores.
  - scores via fp16 hi/lo split: x = h + l (fp16 each), scores = h.h + h.l + l.h
    on TensorE at bf16 rate with ~fp32 accuracy (validated on HW: 2.9e-5 max err).
  - top-8 threshold per row via DVE max8; A = (scores >= thr) as fp16 0/1.
  - A transposed 128x128 on TensorE; msg = A^T-matmuls against fp16 x.
  - tail: blended = msg*(1-mix)/deg + mix*x (host pre-scales x rows by mix),
    exact-erf Gelu on ScalarE, * scale on DVE.
"""

import sys
import types

try:
    import concourse  # provided by the runtime environment (axon site)
except ImportError:
    sys.path.insert(0, "/opt/trn_rl_repo")

# run_bass_kernel_spmd imports antenv.axon_hooks when BASS_TRACE is set; the
# module is absent in this image, so provide a no-trace stub.
try:
    import antenv.axon_hooks  # noqa: F401
except ImportError:
    _m = types.ModuleType("antenv.axon_hooks")
    _m.get_axon_ntff_profile_hook = lambda: None
    sys.modules["antenv.axon_hooks"] = _m

import numpy as np
import ml_dtypes

import concourse.bacc as bacc
import concourse.tile as tile
import concourse.mybir as mybir
from concourse.bass_utils import run_bass_kernel_spmd

F32 = mybir.dt.float32
F16 = mybir.dt.float16
AF = mybir.ActivationFunctionType
ALU = mybir.AluOpType
AX = mybir.AxisListType

B, T, D, K = 4, 2048, 1024, 8
NCORES = 8
SLOTS = 8
NW = [16 - 2 * g for g in range(SLOTS)]  # slot widths in 128-blocks
BIG = np.float32(3e38)
NEG_CLAMP = -1e30

_cache = {}


def _chunks(w):
    """split [0, w) into <=512 pieces"""
    out = []
    j = 0
    while j < w:
        n = min(512, w - j)
        out.append((j, n))
        j += n
    return out


def _build_program(repeat=1):
    nc = bacc.Bacc("TRN2", target_bir_lowering=False, debug=False,
                   num_devices=NCORES)

    # ---- DRAM I/O (per-core shapes; SPMD identical program) ----
    # hi/lo fp16 of x^T, d-chunk major: [:, k*T + j] = x[b, perm(j), 128k+p]
    xth_d = nc.declare_dram_parameter("xth", [128, 8 * T], F16, isOutput=False)
    xtl_d = nc.declare_dram_parameter("xtl", [128, 8 * T], F16, isOutput=False)
    # fp16 x natural, j-chunk major: [:, c*D + d] = x[b, perm(128c+p), d]
    xn_d = nc.declare_dram_parameter("xn", [128, 16 * D], F16, isOutput=False)
    # mix * x rows, slot major, fp16 (+ gain/bias applied if nontrivial)
    xr_d = nc.declare_dram_parameter("xr", [128, 8 * D], F16, isOutput=False)
    # causal mask bias for the last 256 columns of each slot
    msk_d = nc.declare_dram_parameter("msk", [128, 256], F32, isOutput=False)
    idt_d = nc.declare_dram_parameter("idt", [128, 128], F16, isOutput=False)
    # per-partition constants: col0 = (1-mix), col1 = scale
    cv_d = nc.declare_dram_parameter("cv", [128, 2], F32, isOutput=False)
    out_d = nc.declare_dram_parameter("out", [8, 128, D], F32, isOutput=True)

    with tile.TileContext(nc) as tc:
        with (
            tc.tile_pool(name="cst", bufs=1) as cst,
            tc.tile_pool(name="sc", bufs=3) as scp,
            tc.tile_pool(name="ap", bufs=3) as app,
            tc.tile_pool(name="atp", bufs=3) as atp,
            tc.tile_pool(name="sm", bufs=3) as sm,
            tc.tile_pool(name="bl", bufs=3) as blp,
            tc.tile_pool(name="ob", bufs=2) as obp,
            tc.tile_pool(name="ps1", bufs=4, space="PSUM") as ps1,
            tc.tile_pool(name="pst", bufs=2, space="PSUM") as pst,
            tc.tile_pool(name="ps2", bufs=2, space="PSUM") as ps2,
        ):
            xth = cst.tile([128, 8 * T], F16, tag="xth")
            xtl = cst.tile([128, 8 * T], F16, tag="xtl")
            xn = cst.tile([128, 16 * D], F16, tag="xn")
            xr = cst.tile([128, 8 * D], F16, tag="xr")
            msk = cst.tile([128, 256], F32, tag="msk")
            idt = cst.tile([128, 128], F16, tag="idt")
            cv = cst.tile([128, 2], F32, tag="cv")
            # small first (starts early), smallest last (short tail)
            order = [6, 0, 2, 4, 1, 3, 5, 7]
            # fine-grained input DMAs so the first matmuls start after ~256KB
            H = T // 2
            for k in range(8):
                for hh in range(2):
                    s0 = k * T + hh * H
                    nc.sync.dma_start(xth[:, s0:s0 + H], xth_d[:, s0:s0 + H])
                    nc.sync.dma_start(xtl[:, s0:s0 + H], xtl_d[:, s0:s0 + H])
            nc.sync.dma_start(msk[:], msk_d[:])
            nc.sync.dma_start(idt[:], idt_d[:])
            nc.sync.dma_start(cv[:], cv_d[:])
            nc.sync.dma_start(xn[:], xn_d[:])
            nc.sync.dma_start(xr[:], xr_d[:])

            for gi in range(SLOTS * repeat):
                g = order[gi % SLOTS]
                is_last = (gi % SLOTS) == SLOTS - 1
                nw = NW[g]
                W = 128 * nw
                # ---- MM1: causal scores row-tile (128, W), fp16 hi/lo x3 ----
                # k-outer so PE can start as soon as the k=0 chunk DMA lands;
                # one psum tile per j-chunk held across the k loop.
                scores = scp.tile([128, T], F32, tag="scores")
                cks = _chunks(W)
                pts = [ps1.tile([128, 512], F32, tag="mm1", name=f"pt{g}_{ci}")
                       for ci in range(len(cks))]
                for k in range(8):
                    qh = xth[:, k * T + W - 128:k * T + W]
                    ql = xtl[:, k * T + W - 128:k * T + W]
                    # qh-group then ql-group: stationary stays loaded across
                    # the chunk sweep (1 ldweights per group per k)
                    for ci, (j0, n) in enumerate(cks):
                        mh = xth[:, k * T + j0:k * T + j0 + n]
                        ml = xtl[:, k * T + j0:k * T + j0 + n]
                        pt = pts[ci]
                        nc.tensor.matmul(pt[:, :n], qh, mh, start=(k == 0),
                                         stop=False)
                        nc.tensor.matmul(pt[:, :n], qh, ml, start=False,
                                         stop=False)
                    for ci, (j0, n) in enumerate(cks):
                        mh = xth[:, k * T + j0:k * T + j0 + n]
                        nc.tensor.matmul(pts[ci][:, :n], ql, mh, start=False,
                                         stop=(k == 7))
                for ci, (j0, n) in enumerate(cks):
                    if j0 + n < W - 256 + 1:
                        nc.vector.tensor_copy(scores[:, j0:j0 + n],
                                              pts[ci][:, :n])
                    else:
                        # copy + causal mask fused for the final 256 columns
                        lo = max(j0, W - 256)
                        if lo > j0:
                            nc.vector.tensor_copy(scores[:, j0:lo],
                                                  pts[ci][:, :lo - j0])
                        nc.vector.tensor_tensor(
                            scores[:, lo:j0 + n],
                            pts[ci][:, lo - j0:n],
                            msk[:, lo - (W - 256):lo - (W - 256) + (j0 + n - lo)],
                            ALU.min)

                # ---- top-8 threshold, A, deg ----
                m8 = sm.tile([128, 8], F32, tag="m8")
                nc.vector.max(m8[:], scores[:, :W])
                thr = sm.tile([128, 1], F32, tag="thr")
                nc.vector.tensor_scalar_max(thr[:], m8[:, 7:8], NEG_CLAMP)
                A = app.tile([128, T], F16, tag="A")
                nc.vector.tensor_scalar(A[:, :W], scores[:, :W], thr[:], None,
                                        op0=ALU.is_ge)
                v8 = sm.tile([128, 8], F32, tag="v8")
                nc.vector.tensor_scalar(v8[:], m8[:], NEG_CLAMP, None,
                                        op0=ALU.is_ge)
                deg = sm.tile([128, 1], F32, tag="deg")
                nc.vector.tensor_reduce(deg[:], v8[:], AX.X, ALU.add)
                rd = sm.tile([128, 1], F32, tag="rd")
                nc.vector.reciprocal(rd[:], deg[:])
                sv = sm.tile([128, 1], F32, tag="sv")  # (1-mix)/deg
                nc.vector.tensor_tensor(sv[:], rd[:], cv[:, 0:1], ALU.mult)

                # ---- transpose A blocks ----
                at = atp.tile([128, 16 * 128], F16, tag="at")
                for c in range(nw):
                    tp = pst.tile([128, 128], F16, tag="tp")
                    nc.tensor.transpose(tp[:], A[:, c * 128:(c + 1) * 128],
                                        idt[:])
                    nc.scalar.copy(at[:, c * 128:(c + 1) * 128], tp[:])

                # ---- MM2 + tail ----
                # c-outer: each at[c] stationary loads once for both d-halves
                outsb = obp.tile([128, D], F32, tag="outsb")
                pms = [ps2.tile([128, 512], F32, tag="mm2", name=f"pm{g}_{dh}")
                       for dh in range(2)]
                if is_last:
                    # dh-outer: pm0 finishes early so the tail chain starts
                    # while pm1 still accumulates
                    for dh in range(2):
                        for c in range(nw):
                            nc.tensor.matmul(
                                pms[dh][:], at[:, c * 128:(c + 1) * 128],
                                xn[:, c * D + dh * 512:c * D + dh * 512 + 512],
                                start=(c == 0), stop=(c == nw - 1))
                else:
                    for c in range(nw):
                        for dh in range(2):
                            nc.tensor.matmul(
                                pms[dh][:], at[:, c * 128:(c + 1) * 128],
                                xn[:, c * D + dh * 512:c * D + dh * 512 + 512],
                                start=(c == 0), stop=(c == nw - 1))
                # last slot: 4 fine pieces on alternating hwdge rings so the
                # post-matmul exposure (chain + DMA fixed path) is minimal
                P = 256 if is_last else 512
                for pi, p0 in enumerate(range(0, D, P)):
                    dh, po = divmod(p0, 512)
                    bl = blp.tile([128, P], F32,
                                  tag="blf" if is_last else "bl")
                    nc.vector.scalar_tensor_tensor(
                        bl[:], pms[dh][:, po:po + P], sv[:],
                        xr[:, g * D + p0:g * D + p0 + P],
                        op0=ALU.mult, op1=ALU.add)
                    hs = slice(p0, p0 + P)
                    nc.scalar.activation(outsb[:, hs], bl[:], AF.Gelu)
                    nc.vector.tensor_scalar_mul(outsb[:, hs], outsb[:, hs],
                                                cv[:, 1:2])
                    dma_eng = nc.scalar if (is_last and pi % 2) else nc.sync
                    dma_eng.dma_start(out_d[g][:, hs], outsb[:, hs])

    nc.finalize()
    return nc


def _f16_split(a):
    h = a.astype(np.float16)
    l = (a - h.astype(np.float32)).astype(np.float16)
    return h, l


def _prep_inputs(x, gain, bias, log_mix, log_scale):
    """Build the 8 per-core input maps."""
    x = np.asarray(x, dtype=np.float32)
    gain = np.asarray(gain, dtype=np.float32)
    bias = np.asarray(bias, dtype=np.float32)
    mix = np.float32(1.0) / (np.float32(1.0) + np.exp(-np.asarray(log_mix, np.float32)))
    scale = np.log1p(np.exp(np.asarray(log_scale, np.float32))).astype(np.float32) + np.float32(0.01)
    one_minus_mix = np.float32(1.0) - mix

    tril = np.tril(np.ones((128, 128), np.bool_))
    tril_bias = np.where(tril, BIG, -BIG).astype(np.float32)
    keep = np.full((128, 128), BIG, np.float32)
    kill = np.full((128, 128), -BIG, np.float32)

    cv = np.zeros((128, 2), np.float32)
    cv[:, 0] = one_minus_mix
    cv[:, 1] = scale

    in_maps = []
    meta = []
    for c in range(NCORES):
        b = c % 4
        grp = c // 4
        if grp == 0:
            perm_blocks = np.arange(16)
            tiles = [15 - 2 * g for g in range(SLOTS)]
            msk = np.concatenate([keep, tril_bias], axis=1)
        else:
            perm_blocks = np.arange(16).reshape(8, 2)[:, ::-1].ravel()
            tiles = [14 - 2 * g for g in range(SLOTS)]
            msk = np.concatenate([kill, tril_bias], axis=1)

        perm_rows = (perm_blocks[:, None] * 128 + np.arange(128)[None, :]).ravel()
        xp = x[b][perm_rows]  # (T, D) permuted rows
        h, l = _f16_split(xp)
        # xth/xtl: (128, 8*T), chunk k = x^T[128k:128k+128, :]
        xth = np.ascontiguousarray(
            h.T.reshape(8, 128, T).transpose(1, 0, 2).reshape(128, 8 * T))
        xtl = np.ascontiguousarray(
            l.T.reshape(8, 128, T).transpose(1, 0, 2).reshape(128, 8 * T))
        # xn: (128, 16*D), chunk c = (x*gain)[perm rows 128c:128c+128, :]
        # (gain folded in so msg*gain comes out of MM2; exact no-op when gain=1)
        xng = (xp * gain[None, :]).astype(np.float16)
        xn = np.ascontiguousarray(
            xng.reshape(16, 128, D).transpose(1, 0, 2).reshape(128, 16 * D))
        # xr: (128, 8*D) slot-major mix*gain*x + bias (true row order), fp16
        xr = np.empty((128, 8 * D), np.float16)
        for g in range(SLOTS):
            r = 128 * tiles[g]
            xr[:, g * D:(g + 1) * D] = ((mix * gain[None, :]) * x[b, r:r + 128, :] + bias[None, :]).astype(np.float16)
        in_maps.append({
            "xth": xth, "xtl": xtl, "xn": xn,
            "xr": xr, "msk": msk,
            "idt": np.eye(128, dtype=np.float16),
            "cv": cv,
        })
        meta.append((b, tiles))
    return in_maps, meta


def kernel(x, gain, bias, log_mix, log_scale):
    if "nc" not in _cache:
        _cache["nc"] = _build_program()
    nc = _cache["nc"]
    in_maps, meta = _prep_inputs(x, gain, bias, log_mix, log_scale)
    res = run_bass_kernel_spmd(nc, in_maps, core_ids=list(range(NCORES)))
    y = np.empty((B, T, D), np.float32)
    for c in range(NCORES):
        b, tiles = meta[c]
        o = res.results[c]["out"]  # (8, 128, D)
        for g in range(SLOTS):
            r = 128 * tiles[g]
            y[b, r:r + 128, :] = o[g]
    return y



# revision 2
# speedup vs baseline: 1.8051x; 1.8051x over previous
"""Trainium2 Bass kernel for causal top-K GNN message passing.

reference semantics (B=4, T=2048, D=1024, K=8):
    scores = x @ x^T per batch, causal (j <= i)
    A[i,j] = 1 iff j among top-8 causal scores of row i
    msg    = (A @ x) / deg
    out    = gelu(mix*x + (1-mix)*msg) * scale       (gain=*, bias=+ general)

Strategy (8 NeuronCores, SPMD single program):
  - core c handles batch b = c % 4; cores 0-3 take row-tiles t = 15-2g
    (slot g = 0..7), cores 4-7 take t = 14-2g.
  - slot g is compiled for causal width W_g = 128*(16-2g) columns; cores 4-7
    use a per-core pair-swapped row-block permutation of the key/value axis so
    their row-tile lands in the last 128 columns of the slot's width. All
    per-core variation lives in the host-prepared input data; the device
    program is identical across cores.
  - This backend executes instructions serially at a roughly flat cost per
    instruction (matmul ~60-80us, DVE-f32 ~39us, ACT ~100us, DMA ~115us,
    PE-transpose ~173us), so the kernel minimizes weighted instruction count:
    * scores in ONE fp32 matmul per (k-chunk, 512-col chunk): 160 calls/iter
      (vs 480 for fp16 hi/lo x3), accumulated straight into a 4-bank PSUM
      tile that the top-8 DVE ops read directly (no SBUF staging copies).
    * A = (scores >= thr8) as fp32; A^T per 128-block via REGULAR fp32 matmul
      against the identity (cheaper than the transpose instruction), packed
      4 blocks per PSUM bank so one DVE copy drains 4 blocks.
    * msg via fp32 matmuls A^T-block @ x-natural (512-wide): 2*nw per slot.
    * tail: one 1024-wide scalar_tensor_tensor (blend+deg-divide), one Gelu
      (fp16 out), one output DMA per slot; the final *scale is applied on the
      host after gather (exactly, in fp32).
"""

import sys
import types

try:
    import concourse  # provided by the runtime environment (axon site)
except ImportError:
    sys.path.insert(0, "/opt/trn_rl_repo")

# run_bass_kernel_spmd imports antenv.axon_hooks when BASS_TRACE is set; the
# module is absent in this image, so provide a no-trace stub.
try:
    import antenv.axon_hooks  # noqa: F401
except ImportError:
    _m = types.ModuleType("antenv.axon_hooks")
    _m.get_axon_ntff_profile_hook = lambda: None
    sys.modules["antenv.axon_hooks"] = _m

import numpy as np

import concourse.bacc as bacc
import concourse.tile as tile
import concourse.mybir as mybir
from concourse.bass_utils import run_bass_kernel_spmd

F32 = mybir.dt.float32
F16 = mybir.dt.float16
AF = mybir.ActivationFunctionType
ALU = mybir.AluOpType
AX = mybir.AxisListType

B, T, D, K = 4, 2048, 1024, 8
NCORES = 8
SLOTS = 8
NW = [16 - 2 * g for g in range(SLOTS)]  # slot widths in 128-blocks
BIG = np.float32(3e38)
NEG_CLAMP = -1e30

_cache = {}


def _chunks(w):
    """split [0, w) into <=512 pieces"""
    out = []
    j = 0
    while j < w:
        n = min(512, w - j)
        out.append((j, n))
        j += n
    return out


def _build_program(repeat=1):
    nc = bacc.Bacc("TRN2", target_bir_lowering=False, debug=False,
                   num_devices=NCORES)

    # ---- DRAM I/O (per-core shapes; SPMD identical program) ----
    # fp32 x^T, d-chunk major: [:, k*T + j] = x[b, perm(j), 128k+p]
    xt_d = nc.declare_dram_parameter("xt", [128, 8 * T], F32, isOutput=False)
    # fp32 x natural (gain folded), j-chunk major:
    #   [:, c*D + d] = (x*gain)[b, perm(128c+p), d]
    xn_d = nc.declare_dram_parameter("xn", [128, 16 * D], F32, isOutput=False)
    # mix*gain*x + bias rows, slot major, fp32 (true row order)
    xr_d = nc.declare_dram_parameter("xr", [128, 8 * D], F32, isOutput=False)
    # causal mask bias for the last 256 columns of each slot
    msk_d = nc.declare_dram_parameter("msk", [128, 256], F32, isOutput=False)
    idt_d = nc.declare_dram_parameter("idt", [128, 128], F32, isOutput=False)
    # per-partition constants: col0 = (1-mix)
    cv_d = nc.declare_dram_parameter("cv", [128, 2], F32, isOutput=False)
    out_d = nc.declare_dram_parameter("out", [8, 128, D], F16, isOutput=True)

    with tile.TileContext(nc) as tc:
        with (
            tc.tile_pool(name="cst", bufs=1) as cst,
            tc.tile_pool(name="ap", bufs=1) as app,
            tc.tile_pool(name="atp", bufs=1) as atp,
            tc.tile_pool(name="sm", bufs=1) as sm,
            tc.tile_pool(name="bl", bufs=1) as blp,
            tc.tile_pool(name="ob", bufs=1) as obp,
            tc.tile_pool(name="psS", bufs=1, space="PSUM") as psS_p,
            tc.tile_pool(name="psT", bufs=1, space="PSUM") as psT_p,
            tc.tile_pool(name="psM", bufs=1, space="PSUM") as psM_p,
        ):
            xt = cst.tile([128, 8 * T], F32, tag="xt")
            xn = cst.tile([128, 16 * D], F32, tag="xn")
            xr = cst.tile([128, 8 * D], F32, tag="xr")
            msk = cst.tile([128, 256], F32, tag="msk")
            idt = cst.tile([128, 128], F32, tag="idt")
            cv = cst.tile([128, 2], F32, tag="cv")
            nc.sync.dma_start(xt[:], xt_d[:])
            nc.sync.dma_start(xn[:], xn_d[:])
            nc.sync.dma_start(xr[:], xr_d[:])
            nc.sync.dma_start(msk[:], msk_d[:])
            nc.sync.dma_start(idt[:], idt_d[:])
            nc.sync.dma_start(cv[:], cv_d[:])

            psS = psS_p.tile([128, 2048], F32, tag="psS")   # 4 banks: scores
            psT = psT_p.tile([128, 512], F32, tag="psT")    # 1 bank: A^T x4
            pm = psM_p.tile([128, 1024], F32, tag="pm")     # 2 banks: msg

            for gi in range(SLOTS * repeat):
                g = gi % SLOTS
                nw = NW[g]
                W = 128 * nw

                # ---- MM1: causal scores row-tile (128, W), fp32 ----
                for j0, n in _chunks(W):
                    for k in range(8):
                        q = xt[:, k * T + W - 128:k * T + W]
                        m = xt[:, k * T + j0:k * T + j0 + n]
                        nc.tensor.matmul(psS[:, j0:j0 + n], q, m,
                                         start=(k == 0), stop=(k == 7))

                # causal mask on the last 256 columns (in-place on PSUM)
                nc.vector.tensor_tensor(psS[:, W - 256:W], psS[:, W - 256:W],
                                        msk[:], ALU.min)

                # ---- top-8 threshold, A, deg (straight from PSUM) ----
                m8 = sm.tile([128, 8], F32, tag="m8")
                nc.vector.max(m8[:], psS[:, :W])
                thr = sm.tile([128, 1], F32, tag="thr")
                nc.vector.tensor_scalar_max(thr[:], m8[:, 7:8], NEG_CLAMP)
                A = app.tile([128, 2048], F32, tag="A")
                nc.vector.tensor_scalar(A[:, :W], psS[:, :W], thr[:], None,
                                        op0=ALU.is_ge)
                v8 = sm.tile([128, 8], F32, tag="v8")
                nc.vector.tensor_scalar(v8[:], m8[:], NEG_CLAMP, None,
                                        op0=ALU.is_ge)
                deg = sm.tile([128, 1], F32, tag="deg")
                nc.vector.tensor_reduce(deg[:], v8[:], AX.X, ALU.add)
                rd = sm.tile([128, 1], F32, tag="rd")
                nc.vector.reciprocal(rd[:], deg[:])
                sv = sm.tile([128, 1], F32, tag="sv")  # (1-mix)/deg
                nc.vector.tensor_tensor(sv[:], rd[:], cv[:, 0:1], ALU.mult)

                # ---- A^T via regular matmul vs identity; 4 blocks per bank,
                # one DVE copy drains each group of 4 ----
                at = atp.tile([128, 2048], F32, tag="at")
                for c in range(nw):
                    s4 = c % 4
                    nc.tensor.matmul(psT[:, s4 * 128:(s4 + 1) * 128],
                                     A[:, c * 128:(c + 1) * 128], idt[:],
                                     start=True, stop=True)
                    if s4 == 3 or c == nw - 1:
                        c0 = c - s4
                        nc.vector.tensor_copy(
                            at[:, c0 * 128:(c + 1) * 128],
                            psT[:, :(s4 + 1) * 128])

                # ---- MM2: msg = A^T-blocks @ x-natural ----
                for c in range(nw):
                    for h in range(2):
                        nc.tensor.matmul(
                            pm[:, h * 512:(h + 1) * 512],
                            at[:, c * 128:(c + 1) * 128],
                            xn[:, c * D + h * 512:c * D + h * 512 + 512],
                            start=(c == 0), stop=(c == nw - 1))

                # ---- tail: blend, gelu (fp16 out), DMA ----
                bl = blp.tile([128, D], F32, tag="bl")
                nc.vector.scalar_tensor_tensor(
                    bl[:], pm[:], sv[:], xr[:, g * D:(g + 1) * D],
                    op0=ALU.mult, op1=ALU.add)
                outsb = obp.tile([128, D], F16, tag="outsb")
                nc.scalar.activation(outsb[:], bl[:], AF.Gelu)
                nc.sync.dma_start(out_d[g][:], outsb[:])

    nc.finalize()
    return nc


def _prep_inputs(x, gain, bias, log_mix, log_scale):
    """Build the 8 per-core input maps."""
    x = np.asarray(x, dtype=np.float32)
    gain = np.asarray(gain, dtype=np.float32)
    bias = np.asarray(bias, dtype=np.float32)
    mix = np.float32(1.0) / (np.float32(1.0) + np.exp(-np.asarray(log_mix, np.float32)))
    scale = np.log1p(np.exp(np.asarray(log_scale, np.float32))).astype(np.float32) + np.float32(0.01)
    one_minus_mix = np.float32(1.0) - mix

    tril = np.tril(np.ones((128, 128), np.bool_))
    tril_bias = np.where(tril, BIG, -BIG).astype(np.float32)
    keep = np.full((128, 128), BIG, np.float32)
    kill = np.full((128, 128), -BIG, np.float32)

    cv = np.zeros((128, 2), np.float32)
    cv[:, 0] = one_minus_mix

    in_maps = []
    meta = []
    for c in range(NCORES):
        b = c % 4
        grp = c // 4
        if grp == 0:
            perm_blocks = np.arange(16)
            tiles = [15 - 2 * g for g in range(SLOTS)]
            msk = np.concatenate([keep, tril_bias], axis=1)
        else:
            perm_blocks = np.arange(16).reshape(8, 2)[:, ::-1].ravel()
            tiles = [14 - 2 * g for g in range(SLOTS)]
            msk = np.concatenate([kill, tril_bias], axis=1)

        perm_rows = (perm_blocks[:, None] * 128 + np.arange(128)[None, :]).ravel()
        xp = x[b][perm_rows]  # (T, D) permuted rows
        # xt: (128, 8*T), chunk k = x^T[128k:128k+128, :]
        xt = np.ascontiguousarray(
            xp.T.reshape(8, 128, T).transpose(1, 0, 2).reshape(128, 8 * T))
        # xn: (128, 16*D), chunk c = (x*gain)[perm rows 128c:128c+128, :]
        xng = xp * gain[None, :]
        xn = np.ascontiguousarray(
            xng.reshape(16, 128, D).transpose(1, 0, 2).reshape(128, 16 * D))
        # xr: (128, 8*D) slot-major mix*gain*x + bias (true row order)
        xr = np.empty((128, 8 * D), np.float32)
        for g in range(SLOTS):
            r = 128 * tiles[g]
            xr[:, g * D:(g + 1) * D] = (mix * gain[None, :]) * x[b, r:r + 128, :] + bias[None, :]
        in_maps.append({
            "xt": xt, "xn": xn, "xr": xr, "msk": msk,
            "idt": np.eye(128, dtype=np.float32),
            "cv": cv,
        })
        meta.append((b, tiles, scale))
    return in_maps, meta


def kernel(x, gain, bias, log_mix, log_scale):
    if "nc" not in _cache:
        _cache["nc"] = _build_program()
    nc = _cache["nc"]
    in_maps, meta = _prep_inputs(x, gain, bias, log_mix, log_scale)
    res = run_bass_kernel_spmd(nc, in_maps, core_ids=list(range(NCORES)))
    y = np.empty((B, T, D), np.float32)
    for c in range(NCORES):
        b, tiles, scale = meta[c]
        o = res.results[c]["out"].astype(np.float32) * scale  # (8, 128, D)
        for g in range(SLOTS):
            r = 128 * tiles[g]
            y[b, r:r + 128, :] = o[g]
    return y


# revision 3
# speedup vs baseline: 1.8076x; 1.0014x over previous
"""Trainium2 Bass kernel for causal top-K GNN message passing.

reference semantics (B=4, T=2048, D=1024, K=8):
    scores = x @ x^T per batch, causal (j <= i)
    A[i,j] = 1 iff j among top-8 causal scores of row i
    msg    = (A @ x) / deg
    out    = gelu(mix*x + (1-mix)*msg) * scale       (gain=*, bias=+ general)

Strategy (8 NeuronCores, SPMD single program):
  - core c handles batch b = c % 4; cores 0-3 take row-tiles t = 15-2g
    (slot g = 0..7), cores 4-7 take t = 14-2g.
  - slot g is compiled for causal width W_g = 128*(16-2g) columns; cores 4-7
    use a per-core pair-swapped row-block permutation of the key/value axis so
    their row-tile lands in the last 128 columns of the slot's width. All
    per-core variation lives in the host-prepared input data; the device
    program is identical across cores.
  - This backend executes instructions serially at a roughly flat cost per
    instruction (matmul ~60-90us, DVE-f32 ~20-49us, ACT ~100-126us, DMA
    ~15us+57us/MB, PE-transpose-instr ~173us, cross-engine sync ~50-100us),
    so the kernel minimizes weighted instruction count and sync edges:
    * scores in ONE fp32 matmul per (k-chunk, 512-col chunk): 160 calls/iter,
      k-outer so the accumulation chains interleave across PSUM banks; the
      top-8 DVE ops read the PSUM scores directly (no staging copies).
    * deg is deterministic (min(row+1, 8)), so (1-mix)/deg ships as a host
      precomputed per-partition constant -- no v8/deg/reciprocal on device.
    * A = (scores >= thr8) fp32; A^T per 128-block via REGULAR fp32 matmul
      against the identity (cheaper than the transpose instruction), written
      back into the then-dead scores PSUM banks; one DVE copy drains 4
      blocks, casting to fp16.
    * msg via fp16 matmuls A^T-block @ x-natural (512-wide), 2*nw per slot.
    * tail: one 1024-wide scalar_tensor_tensor per slot (blend + deg-divide,
      fp16 out), then a single 8192-wide Gelu and 8 output DMAs per
      iteration; the final *scale is applied on the host after gather.
"""

import sys
import types

try:
    import concourse  # provided by the runtime environment (axon site)
except ImportError:
    sys.path.insert(0, "/opt/trn_rl_repo")

# run_bass_kernel_spmd imports antenv.axon_hooks when BASS_TRACE is set; the
# module is absent in this image, so provide a no-trace stub.
try:
    import antenv.axon_hooks  # noqa: F401
except ImportError:
    _m = types.ModuleType("antenv.axon_hooks")
    _m.get_axon_ntff_profile_hook = lambda: None
    sys.modules["antenv.axon_hooks"] = _m

import numpy as np

import concourse.bacc as bacc
import concourse.tile as tile
import concourse.mybir as mybir
from concourse.bass_utils import run_bass_kernel_spmd

F32 = mybir.dt.float32
F16 = mybir.dt.float16
AF = mybir.ActivationFunctionType
ALU = mybir.AluOpType
AX = mybir.AxisListType

B, T, D, K = 4, 2048, 1024, 8
NCORES = 8
SLOTS = 8
NW = [16 - 2 * g for g in range(SLOTS)]  # slot widths in 128-blocks
BIG = np.float32(3e38)
NEG_CLAMP = -1e30

_cache = {}


def _chunks(w):
    """split [0, w) into <=512 pieces"""
    out = []
    j = 0
    while j < w:
        n = min(512, w - j)
        out.append((j, n))
        j += n
    return out


def _build_program(repeat=1):
    nc = bacc.Bacc("TRN2", target_bir_lowering=False, debug=False,
                   num_devices=NCORES)

    # ---- DRAM I/O (per-core shapes; SPMD identical program) ----
    # fp32 x^T, d-chunk major: [:, k*T + j] = x[b, perm(j), 128k+p]
    xt_d = nc.declare_dram_parameter("xt", [128, 8 * T], F32, isOutput=False)
    # fp16 x natural (gain folded), j-chunk major:
    #   [:, c*D + d] = (x*gain)[b, perm(128c+p), d]
    xn_d = nc.declare_dram_parameter("xn", [128, 16 * D], F16, isOutput=False)
    # mix*gain*x + bias rows, slot major, fp16 (true row order)
    xr_d = nc.declare_dram_parameter("xr", [128, 8 * D], F16, isOutput=False)
    # causal mask bias for the last 256 columns of each slot
    msk_d = nc.declare_dram_parameter("msk", [128, 256], F32, isOutput=False)
    idt_d = nc.declare_dram_parameter("idt", [128, 128], F32, isOutput=False)
    # per-partition constants: col g = (1-mix)/deg(core, slot g, partition)
    sv_d = nc.declare_dram_parameter("sv", [128, 8], F32, isOutput=False)
    out_d = nc.declare_dram_parameter("out", [8, 128, D], F16, isOutput=True)

    with tile.TileContext(nc) as tc:
        with (
            tc.tile_pool(name="cst", bufs=1) as cst,
            tc.tile_pool(name="ap", bufs=1) as app,
            tc.tile_pool(name="atp", bufs=1) as atp,
            tc.tile_pool(name="sm", bufs=1) as sm,
            tc.tile_pool(name="bl", bufs=1) as blp,
            tc.tile_pool(name="ob", bufs=1) as obp,
            tc.tile_pool(name="psS", bufs=1, space="PSUM") as psS_p,
            tc.tile_pool(name="psM", bufs=1, space="PSUM") as psM_p,
        ):
            xt = cst.tile([128, 8 * T], F32, tag="xt")
            xn = cst.tile([128, 16 * D], F16, tag="xn")
            xr = cst.tile([128, 8 * D], F16, tag="xr")
            msk = cst.tile([128, 256], F32, tag="msk")
            idt = cst.tile([128, 128], F32, tag="idt")
            sv = cst.tile([128, 8], F32, tag="sv")
            nc.sync.dma_start(xt[:], xt_d[:])
            nc.sync.dma_start(xn[:], xn_d[:])
            nc.sync.dma_start(xr[:], xr_d[:])
            nc.sync.dma_start(msk[:], msk_d[:])
            nc.sync.dma_start(idt[:], idt_d[:])
            nc.sync.dma_start(sv[:], sv_d[:])

            psS = psS_p.tile([128, 2048], F32, tag="psS")   # 4 banks
            pm = psM_p.tile([128, 1024], F32, tag="pm")     # 2 banks

            for gi in range(SLOTS * repeat):
                g = gi % SLOTS
                nw = NW[g]
                W = 128 * nw
                cks = _chunks(W)

                # ---- MM1: causal scores row-tile (128, W), fp32; k-outer so
                # chains interleave across the chunk banks ----
                for k in range(8):
                    q = xt[:, k * T + W - 128:k * T + W]
                    for j0, n in cks:
                        nc.tensor.matmul(psS[:, j0:j0 + n], q,
                                         xt[:, k * T + j0:k * T + j0 + n],
                                         start=(k == 0), stop=(k == 7))

                # causal mask on the last 256 columns (in-place on PSUM)
                nc.vector.tensor_tensor(psS[:, W - 256:W], psS[:, W - 256:W],
                                        msk[:], ALU.min)

                # ---- top-8 threshold, A (straight from PSUM) ----
                m8 = sm.tile([128, 8], F32, tag="m8")
                nc.vector.max(m8[:], psS[:, :W])
                thr = sm.tile([128, 1], F32, tag="thr")
                nc.vector.tensor_scalar_max(thr[:], m8[:, 7:8], NEG_CLAMP)
                A = app.tile([128, 2048], F32, tag="A")
                nc.vector.tensor_scalar(A[:, :W], psS[:, :W], thr[:], None,
                                        op0=ALU.is_ge)

                # ---- A^T via regular fp32 matmul vs identity, written into
                # the dead scores banks; one fp16 copy drains 4 blocks ----
                at = atp.tile([128, 2048], F16, tag="at")
                for c in range(nw):
                    nc.tensor.matmul(psS[:, c * 128:(c + 1) * 128],
                                     A[:, c * 128:(c + 1) * 128], idt[:],
                                     start=True, stop=True)
                for b4 in range(0, nw, 4):
                    hi = min(b4 + 4, nw)
                    nc.vector.tensor_copy(at[:, b4 * 128:hi * 128],
                                          psS[:, b4 * 128:hi * 128])

                # ---- MM2: msg = A^T-blocks @ x-natural (fp16) ----
                for c in range(nw):
                    for h in range(2):
                        nc.tensor.matmul(
                            pm[:, h * 512:(h + 1) * 512],
                            at[:, c * 128:(c + 1) * 128],
                            xn[:, c * D + h * 512:c * D + h * 512 + 512],
                            start=(c == 0), stop=(c == nw - 1))

                # ---- blend (deg-divide via precomputed sv), fp16 out ----
                if g == 0:
                    blall = blp.tile([128, 8 * D], F16, tag="blall",
                                     name=f"blall{gi}")
                nc.vector.scalar_tensor_tensor(
                    blall[:, g * D:(g + 1) * D], pm[:], sv[:, g:g + 1],
                    xr[:, g * D:(g + 1) * D], op0=ALU.mult, op1=ALU.add)

                # ---- once per iteration: single wide Gelu + output DMAs ----
                if g == SLOTS - 1:
                    outsb = obp.tile([128, 8 * D], F16, tag="outsb")
                    nc.scalar.activation(outsb[:], blall[:], AF.Gelu)
                    for gg in range(SLOTS):
                        nc.sync.dma_start(out_d[gg][:],
                                          outsb[:, gg * D:(gg + 1) * D])

    nc.finalize()
    return nc


def _prep_inputs(x, gain, bias, log_mix, log_scale):
    """Build the 8 per-core input maps."""
    x = np.asarray(x, dtype=np.float32)
    gain = np.asarray(gain, dtype=np.float32)
    bias = np.asarray(bias, dtype=np.float32)
    mix = np.float32(1.0) / (np.float32(1.0) + np.exp(-np.asarray(log_mix, np.float32)))
    scale = np.log1p(np.exp(np.asarray(log_scale, np.float32))).astype(np.float32) + np.float32(0.01)
    one_minus_mix = np.float32(1.0) - mix

    tril = np.tril(np.ones((128, 128), np.bool_))
    tril_bias = np.where(tril, BIG, -BIG).astype(np.float32)
    keep = np.full((128, 128), BIG, np.float32)
    kill = np.full((128, 128), -BIG, np.float32)

    in_maps = []
    meta = []
    for c in range(NCORES):
        b = c % 4
        grp = c // 4
        if grp == 0:
            perm_blocks = np.arange(16)
            tiles = [15 - 2 * g for g in range(SLOTS)]
            msk = np.concatenate([keep, tril_bias], axis=1)
        else:
            perm_blocks = np.arange(16).reshape(8, 2)[:, ::-1].ravel()
            tiles = [14 - 2 * g for g in range(SLOTS)]
            msk = np.concatenate([kill, tril_bias], axis=1)

        # sv[p, g] = (1-mix)/deg, deg = min(global_row+1, 8) is deterministic
        sv = np.empty((128, 8), np.float32)
        for g in range(SLOTS):
            rows = 128 * tiles[g] + np.arange(128)
            deg = np.minimum(rows + 1, 8).astype(np.float32)
            sv[:, g] = one_minus_mix / deg

        perm_rows = (perm_blocks[:, None] * 128 + np.arange(128)[None, :]).ravel()
        xp = x[b][perm_rows]  # (T, D) permuted rows
        # xt: (128, 8*T), chunk k = x^T[128k:128k+128, :]
        xt = np.ascontiguousarray(
            xp.T.reshape(8, 128, T).transpose(1, 0, 2).reshape(128, 8 * T))
        # xn: (128, 16*D) fp16, chunk c = (x*gain)[perm rows 128c:128c+128, :]
        xng = (xp * gain[None, :]).astype(np.float16)
        xn = np.ascontiguousarray(
            xng.reshape(16, 128, D).transpose(1, 0, 2).reshape(128, 16 * D))
        # xr: (128, 8*D) fp16 slot-major mix*gain*x + bias (true row order)
        xr = np.empty((128, 8 * D), np.float16)
        for g in range(SLOTS):
            r = 128 * tiles[g]
            xr[:, g * D:(g + 1) * D] = ((mix * gain[None, :]) * x[b, r:r + 128, :] + bias[None, :]).astype(np.float16)
        in_maps.append({
            "xt": xt, "xn": xn, "xr": xr, "msk": msk,
            "idt": np.eye(128, dtype=np.float32),
            "sv": sv,
        })
        meta.append((b, tiles, scale))
    return in_maps, meta


def kernel(x, gain, bias, log_mix, log_scale):
    if "nc" not in _cache:
        _cache["nc"] = _build_program()
    nc = _cache["nc"]
    in_maps, meta = _prep_inputs(x, gain, bias, log_mix, log_scale)
    res = run_bass_kernel_spmd(nc, in_maps, core_ids=list(range(NCORES)))
    y = np.empty((B, T, D), np.float32)
    for c in range(NCORES):
        b, tiles, scale = meta[c]
        o = res.results[c]["out"].astype(np.float32) * scale  # (8, 128, D)
        for g in range(SLOTS):
            r = 128 * tiles[g]
            y[b, r:r + 128, :] = o[g]
    return y


# revision 8
# speedup vs baseline: 2.6845x; 1.4851x over previous
"""Trainium2 Bass kernel for causal top-K GNN message passing.

reference semantics (B=4, T=2048, D=1024, K=8):
    scores = x @ x^T per batch, causal (j <= i)
    A[i,j] = 1 iff j among top-8 causal scores of row i
    msg    = (A @ x) / deg
    out    = gelu(mix*x + (1-mix)*msg) * scale       (gain=*, bias=+ general)

Strategy (8 NeuronCores, SPMD single program):
  - core c handles batch b = c % 4; cores 0-3 take row-tiles t = 15-2g
    (slot g = 0..7), cores 4-7 take t = 14-2g.
  - slot g is compiled for causal width W_g = 128*(16-2g) columns; cores 4-7
    use a per-core pair-swapped row-block permutation of the key/value axis so
    their row-tile lands in the last 128 columns of the slot's width. All
    per-core variation lives in the host-prepared input data; the device
    program is identical across cores.
  - This backend executes instructions serially at a roughly flat cost per
    instruction (matmul ~60-90us, DVE-f32 ~20-49us, ACT ~100-126us, DMA
    ~15us+57us/MB, PE-transpose-instr ~173us, cross-engine sync ~50-100us),
    so the kernel minimizes weighted instruction count and sync edges:
    * scores in ONE fp32 matmul per (k-chunk, 512-col chunk): 160 calls/iter,
      k-outer so the accumulation chains interleave across PSUM banks; the
      top-8 DVE ops read the PSUM scores directly (no staging copies).
    * deg is deterministic (min(row+1, 8)), so (1-mix)/deg ships as a host
      precomputed per-partition constant -- no v8/deg/reciprocal on device.
    * A = (scores >= thr8) fp32; A^T per 128-block via REGULAR fp32 matmul
      against the identity (cheaper than the transpose instruction), written
      back into the then-dead scores PSUM banks; one DVE copy drains 4
      blocks, casting to fp16.
    * msg via fp16 matmuls A^T-block @ x-natural (512-wide), 2*nw per slot.
    * tail: one 1024-wide scalar_tensor_tensor per slot (blend + deg-divide,
      fp16 out), then a single 8192-wide Gelu and 8 output DMAs per
      iteration; the final *scale is applied on the host after gather.
"""

import sys
import types

try:
    import concourse  # provided by the runtime environment (axon site)
except ImportError:
    sys.path.insert(0, "/opt/trn_rl_repo")

# run_bass_kernel_spmd imports antenv.axon_hooks when BASS_TRACE is set; the
# module is absent in this image, so provide a no-trace stub.
try:
    import antenv.axon_hooks  # noqa: F401
except ImportError:
    _m = types.ModuleType("antenv.axon_hooks")
    _m.get_axon_ntff_profile_hook = lambda: None
    sys.modules["antenv.axon_hooks"] = _m

import numpy as np

import concourse.bacc as bacc
import concourse.tile as tile
import concourse.mybir as mybir
from concourse.bass_utils import run_bass_kernel_spmd

F32 = mybir.dt.float32
F16 = mybir.dt.float16
AF = mybir.ActivationFunctionType
ALU = mybir.AluOpType
AX = mybir.AxisListType

B, T, D, K = 4, 2048, 1024, 8
NCORES = 8
SLOTS = 8
NW = [16 - 2 * g for g in range(SLOTS)]  # slot widths in 128-blocks
BIG = np.float32(3e38)
NEG_CLAMP = -1e30

_cache = {}


def _chunks(w):
    """split [0, w) into <=512 pieces"""
    out = []
    j = 0
    while j < w:
        n = min(512, w - j)
        out.append((j, n))
        j += n
    return out


def _build_program(repeat=1, skip=()):
    nc = bacc.Bacc("TRN2", target_bir_lowering=False, debug=False,
                   num_devices=NCORES)

    # ---- DRAM I/O (per-core shapes; SPMD identical program) ----
    # fp32 x^T, d-chunk major: [:, k*T + j] = x[b, perm(j), 128k+p]
    xt_d = nc.declare_dram_parameter("xt", [128, 8 * T], F32, isOutput=False)
    # fp16 x natural (gain folded), j-chunk major:
    #   [:, c*D + d] = (x*gain)[b, perm(128c+p), d]
    xn_d = nc.declare_dram_parameter("xn", [128, 16 * D], F32, isOutput=False)
    # mix*gain*x + bias rows, slot major, fp16 (true row order)
    xr_d = nc.declare_dram_parameter("xr", [128, 8 * D], F16, isOutput=False)
    # causal mask bias for the last 256 columns of each slot
    msk_d = nc.declare_dram_parameter("msk", [128, 256], F32, isOutput=False)
    idt_d = nc.declare_dram_parameter("idt", [128, 128], F32, isOutput=False)
    # per-partition constants: col g = (1-mix)/deg(core, slot g, partition)
    sv_d = nc.declare_dram_parameter("sv", [128, 8], F32, isOutput=False)
    out_d = nc.declare_dram_parameter("out", [128, 8 * D], F16, isOutput=True)

    with tile.TileContext(nc) as tc:
        with (
            tc.tile_pool(name="cst", bufs=1) as cst,
            tc.tile_pool(name="ap", bufs=1) as app,
            tc.tile_pool(name="atp", bufs=1) as atp,
            tc.tile_pool(name="sm", bufs=1) as sm,
            tc.tile_pool(name="bl", bufs=1) as blp,
            tc.tile_pool(name="ob", bufs=1) as obp,
            tc.tile_pool(name="psS", bufs=1, space="PSUM") as psS_p,
            tc.tile_pool(name="psM", bufs=1, space="PSUM") as psM_p,
        ):
            xt = cst.tile([128, 8 * T], F32, tag="xt")
            xn = cst.tile([128, 16 * D], F32, tag="xn")
            xr = cst.tile([128, 8 * D], F16, tag="xr")
            msk = cst.tile([128, 256], F32, tag="msk")
            idt = cst.tile([128, 128], F32, tag="idt")
            sv = cst.tile([128, 8], F32, tag="sv")
            nc.sync.dma_start(xt[:], xt_d[:])
            nc.sync.dma_start(xn[:], xn_d[:])
            nc.sync.dma_start(xr[:], xr_d[:])
            nc.sync.dma_start(msk[:], msk_d[:])
            nc.sync.dma_start(idt[:], idt_d[:])
            nc.sync.dma_start(sv[:], sv_d[:])

            psS = psS_p.tile([128, 2048], F32, tag="psS")   # 4 banks
            pm = psM_p.tile([128, 1024], F32, tag="pm")     # 2 banks

            # ablation priming: sections that are skipped leave their outputs
            # unwritten; give downstream readers something to read
            if "MM1" in skip:
                for j0 in range(0, 2048, 512):
                    nc.tensor.matmul(psS[:, j0:j0 + 512], xt[:, :128],
                                     xt[:, :512], start=True, stop=True)
            if "MM2" in skip:
                nc.tensor.matmul(pm[:, :512], xt[:, :128], xt[:, :512],
                                 start=True, stop=True)
                nc.tensor.matmul(pm[:, 512:], xt[:, :128], xt[:, :512],
                                 start=True, stop=True)
            prime_A = app.tile([128, 2048], F32, tag="A", name="prime_A") if "Top8" in skip else None
            if prime_A is not None:
                nc.vector.tensor_copy(prime_A[:], xt[:, :2048])
            prime_at = atp.tile([128, 2048], F32, tag="at", name="prime_at") if "Trans" in skip else None
            if prime_at is not None:
                nc.vector.tensor_copy(prime_at[:], xt[:, :2048])
            prime_sm = sm.tile([128, 16], F32, tag="prime_sm", name="prime_sm") if "Top8" in skip else None
            if prime_sm is not None:
                nc.vector.tensor_copy(prime_sm[:], xt[:, :16])

            for gi in range(SLOTS * repeat):
                g = gi % SLOTS
                nw = NW[g]
                W = 128 * nw
                cks = _chunks(W)

                # ---- MM1: causal scores row-tile (128, W), fp32; k-outer so
                # chains interleave across the chunk banks ----
                if "MM1" not in skip:
                    for k in range(8):
                        q = xt[:, k * T + W - 128:k * T + W]
                        for j0, n in cks:
                            nc.tensor.matmul(psS[:, j0:j0 + n], q,
                                             xt[:, k * T + j0:k * T + j0 + n],
                                             start=(k == 0), stop=(k == 7))

                if "Top8" not in skip:
                    # causal mask on the last 256 columns (in-place on PSUM)
                    nc.vector.tensor_tensor(psS[:, W - 256:W],
                                            psS[:, W - 256:W],
                                            msk[:], ALU.min)

                    # ---- top-8 threshold, A (straight from PSUM) ----
                    m8 = sm.tile([128, 8], F32, tag="m8")
                    nc.vector.max(m8[:], psS[:, :W])
                    thr = sm.tile([128, 1], F32, tag="thr")
                    nc.vector.tensor_scalar_max(thr[:], m8[:, 7:8], NEG_CLAMP)
                    A = app.tile([128, 2048], F32, tag="A")
                    nc.vector.tensor_scalar(A[:, :W], psS[:, :W], thr[:],
                                            None, op0=ALU.is_ge)
                else:
                    A = prime_A

                # ---- A^T via regular fp32 matmul vs identity, written into
                # the dead scores banks; one fp16 copy drains 4 blocks ----
                if "Trans" not in skip:
                    at = atp.tile([128, 2048], F32, tag="at")
                    for c in range(nw):
                        nc.tensor.matmul(psS[:, c * 128:(c + 1) * 128],
                                         A[:, c * 128:(c + 1) * 128], idt[:],
                                         start=True, stop=True)
                    for b4 in range(0, nw, 4):
                        hi = min(b4 + 4, nw)
                        nc.vector.tensor_copy(at[:, b4 * 128:hi * 128],
                                              psS[:, b4 * 128:hi * 128])
                else:
                    at = prime_at

                # ---- MM2: msg = A^T-blocks @ x-natural (fp16) ----
                if "MM2" not in skip:
                    for c in range(nw):
                        for h in range(2):
                            nc.tensor.matmul(
                                pm[:, h * 512:(h + 1) * 512],
                                at[:, c * 128:(c + 1) * 128],
                                xn[:, c * D + h * 512:c * D + h * 512 + 512],
                                start=(c == 0), stop=(c == nw - 1))

                if "Tail" not in skip:
                    # ---- blend (deg-divide via precomputed sv), fp16 out ----
                    if g == 0:
                        blall = blp.tile([128, 8 * D], F16, tag="blall",
                                         name=f"blall{gi}")
                    nc.vector.scalar_tensor_tensor(
                        blall[:, g * D:(g + 1) * D], pm[:], sv[:, g:g + 1],
                        xr[:, g * D:(g + 1) * D], op0=ALU.mult, op1=ALU.add)

                    # ---- once per iteration: one wide Gelu + output DMAs ----
                    if g == SLOTS - 1:
                        outsb = obp.tile([128, 8 * D], F16, tag="outsb")
                        nc.scalar.activation(outsb[:], blall[:], AF.Gelu)
                        nc.sync.dma_start(out_d[:], outsb[:])

    nc.finalize()
    return nc


def _prep_inputs(x, gain, bias, log_mix, log_scale):
    """Build the 8 per-core input maps."""
    x = np.asarray(x, dtype=np.float32)
    gain = np.asarray(gain, dtype=np.float32)
    bias = np.asarray(bias, dtype=np.float32)
    mix = np.float32(1.0) / (np.float32(1.0) + np.exp(-np.asarray(log_mix, np.float32)))
    scale = np.log1p(np.exp(np.asarray(log_scale, np.float32))).astype(np.float32) + np.float32(0.01)
    one_minus_mix = np.float32(1.0) - mix

    tril = np.tril(np.ones((128, 128), np.bool_))
    tril_bias = np.where(tril, BIG, -BIG).astype(np.float32)
    keep = np.full((128, 128), BIG, np.float32)
    kill = np.full((128, 128), -BIG, np.float32)

    in_maps = []
    meta = []
    for c in range(NCORES):
        b = c % 4
        grp = c // 4
        if grp == 0:
            perm_blocks = np.arange(16)
            tiles = [15 - 2 * g for g in range(SLOTS)]
            msk = np.concatenate([keep, tril_bias], axis=1)
        else:
            perm_blocks = np.arange(16).reshape(8, 2)[:, ::-1].ravel()
            tiles = [14 - 2 * g for g in range(SLOTS)]
            msk = np.concatenate([kill, tril_bias], axis=1)

        # sv[p, g] = (1-mix)/deg, deg = min(global_row+1, 8) is deterministic
        sv = np.empty((128, 8), np.float32)
        for g in range(SLOTS):
            rows = 128 * tiles[g] + np.arange(128)
            deg = np.minimum(rows + 1, 8).astype(np.float32)
            sv[:, g] = one_minus_mix / deg

        perm_rows = (perm_blocks[:, None] * 128 + np.arange(128)[None, :]).ravel()
        xp = x[b][perm_rows]  # (T, D) permuted rows
        # xt: (128, 8*T), chunk k = x^T[128k:128k+128, :]
        xt = np.ascontiguousarray(
            xp.T.reshape(8, 128, T).transpose(1, 0, 2).reshape(128, 8 * T))
        # xn: (128, 16*D) fp16, chunk c = (x*gain)[perm rows 128c:128c+128, :]
        xng = xp * gain[None, :]
        xn = np.ascontiguousarray(
            xng.reshape(16, 128, D).transpose(1, 0, 2).reshape(128, 16 * D))
        # xr: (128, 8*D) fp16 slot-major mix*gain*x + bias (true row order)
        xr = np.empty((128, 8 * D), np.float16)
        for g in range(SLOTS):
            r = 128 * tiles[g]
            xr[:, g * D:(g + 1) * D] = ((mix * gain[None, :]) * x[b, r:r + 128, :] + bias[None, :]).astype(np.float16)
        in_maps.append({
            "xt": xt, "xn": xn, "xr": xr, "msk": msk,
            "idt": np.eye(128, dtype=np.float32),
            "sv": sv,
        })
        meta.append((b, tiles, scale))
    return in_maps, meta


def kernel(x, gain, bias, log_mix, log_scale):
    if "nc" not in _cache:
        _cache["nc"] = _build_program()
    nc = _cache["nc"]
    in_maps, meta = _prep_inputs(x, gain, bias, log_mix, log_scale)
    res = run_bass_kernel_spmd(nc, in_maps, core_ids=list(range(NCORES)))
    y = np.empty((B, T, D), np.float32)
    for c in range(NCORES):
        b, tiles, scale = meta[c]
        o = res.results[c]["out"].astype(np.float32) * scale  # (128, 8*D)
        for g in range(SLOTS):
            r = 128 * tiles[g]
            y[b, r:r + 128, :] = o[:, g * D:(g + 1) * D]
    return y


# revision 9
# speedup vs baseline: 2.8109x; 1.0471x over previous
"""Trainium2 Bass kernel for causal top-K GNN message passing.

reference semantics (B=4, T=2048, D=1024, K=8):
    scores = x @ x^T per batch, causal (j <= i)
    A[i,j] = 1 iff j among top-8 causal scores of row i
    msg    = (A @ x) / deg
    out    = gelu(mix*x + (1-mix)*msg) * scale       (gain=*, bias=+ general)

Strategy (8 NeuronCores, SPMD single program):
  - core c handles batch b = c % 4; cores 0-3 take row-tiles t = 15-2g
    (slot g = 0..7), cores 4-7 take t = 14-2g.
  - slot g is compiled for causal width W_g = 128*(16-2g) columns; cores 4-7
    use a per-core pair-swapped row-block permutation of the key/value axis so
    their row-tile lands in the last 128 columns of the slot's width. All
    per-core variation lives in the host-prepared input data; the device
    program is identical across cores.
  - This backend executes instructions serially at a roughly flat per-
    instruction cost (matmul ~60-90us, DVE-f32 ~20-49us, ACT ~100-126us,
    small DMA ~15us, cross-engine sync ~50-100us), so the kernel minimizes
    weighted instruction count:
    * scores in ONE fp32 matmul per (k-chunk, 512-col chunk): 160 calls/iter,
      k-outer so the stationary is reused and accumulation chains interleave
      across PSUM banks; the top-8 DVE ops read the PSUM scores directly.
    * top-8 NEIGHBOR INDICES via max8 + max8_index (no thresholding, no 0/1
      adjacency matrix, no transposes, no second matmul): the 8 indices per
      query drive ONE gpsimd dma_gather that pulls all 1024 neighbor rows of
      x straight from HBM, grouped so each query's 8 rows land in its own
      partition. The gather index tile is built with 8 strided 2-byte DMAs
      (position n = r*128 + i lives at idxs[n%16, n//16]) and must be
      replicated into all 8 groups of 16 partitions (HW DGE cores each read
      their own group; CoreSim only reads group 0).
    * msg*deg = sum of the 8 gathered rows = 3 wide DVE adds (pairwise tree).
    * deg is deterministic (min(row+1, 8)), so (1-mix)/deg ships as a host
      precomputed per-partition constant; one 1024-wide scalar_tensor_tensor
      per slot blends msg with mix*gain*x+bias (fp16 out), then a single
      8192-wide Gelu and one output DMA per iteration.
    * rows 0-6 of each batch have fewer than 8 causal candidates; max8_index
      picks masked entries there, so the host overwrites those 28 rows with
      the exact (trivial: msg = causal running mean) fp32 computation. The
      final *scale is also applied on the host after gather.
"""

import sys
import types

try:
    import concourse  # provided by the runtime environment (axon site)
except ImportError:
    sys.path.insert(0, "/opt/trn_rl_repo")

# run_bass_kernel_spmd imports antenv.axon_hooks when BASS_TRACE is set; the
# module is absent in this image, so provide a no-trace stub.
try:
    import antenv.axon_hooks  # noqa: F401
except ImportError:
    _m = types.ModuleType("antenv.axon_hooks")
    _m.get_axon_ntff_profile_hook = lambda: None
    sys.modules["antenv.axon_hooks"] = _m

import numpy as np

import concourse.bacc as bacc
import concourse.tile as tile
import concourse.mybir as mybir
from concourse.bass_utils import run_bass_kernel_spmd

F32 = mybir.dt.float32
F16 = mybir.dt.float16
U16 = mybir.dt.uint16
I16 = mybir.dt.int16
AF = mybir.ActivationFunctionType
ALU = mybir.AluOpType
AX = mybir.AxisListType

B, T, D, K = 4, 2048, 1024, 8
NCORES = 8
SLOTS = 8
NW = [16 - 2 * g for g in range(SLOTS)]  # slot widths in 128-blocks
BIG = np.float32(3e38)

_cache = {}


def _chunks(w):
    """split [0, w) into <=512 pieces"""
    out = []
    j = 0
    while j < w:
        n = min(512, w - j)
        out.append((j, n))
        j += n
    return out


def _build_program(repeat=1):
    nc = bacc.Bacc("TRN2", target_bir_lowering=False, debug=False,
                   num_devices=NCORES)

    # ---- DRAM I/O (per-core shapes; SPMD identical program) ----
    # fp32 x^T, d-chunk major: [:, k*T + j] = x[b, perm(j), 128k+p]
    xt_d = nc.declare_dram_parameter("xt", [128, 8 * T], F32, isOutput=False)
    # fp16 (x*gain) in permuted row order; dma_gather source (stays in HBM)
    xg_d = nc.declare_dram_parameter("xg", [T, D], F16, isOutput=False)
    # mix*gain*x + bias rows, slot major, fp16 (true row order)
    xr_d = nc.declare_dram_parameter("xr", [128, 8 * D], F16, isOutput=False)
    # causal mask bias for the last 256 columns of each slot
    msk_d = nc.declare_dram_parameter("msk", [128, 256], F32, isOutput=False)
    # per-partition constants: col g = (1-mix)/deg(core, slot g, partition)
    sv_d = nc.declare_dram_parameter("sv", [128, 8], F32, isOutput=False)
    out_d = nc.declare_dram_parameter("out", [128, 8 * D], F16, isOutput=True)

    with tile.TileContext(nc) as tc:
        with (
            tc.tile_pool(name="cst", bufs=1) as cst,
            tc.tile_pool(name="sm", bufs=1) as sm,
            tc.tile_pool(name="ixp", bufs=1) as ixp,
            tc.tile_pool(name="gt", bufs=1) as gtp,
            tc.tile_pool(name="bl", bufs=1) as blp,
            tc.tile_pool(name="ob", bufs=1) as obp,
            tc.tile_pool(name="psS", bufs=1, space="PSUM") as psS_p,
        ):
            xt = cst.tile([128, 8 * T], F32, tag="xt")
            xr = cst.tile([128, 8 * D], F16, tag="xr")
            msk = cst.tile([128, 256], F32, tag="msk")
            sv = cst.tile([128, 8], F32, tag="sv")
            nc.sync.dma_start(xt[:], xt_d[:])
            nc.sync.dma_start(xr[:], xr_d[:])
            nc.sync.dma_start(msk[:], msk_d[:])
            nc.sync.dma_start(sv[:], sv_d[:])

            psS = psS_p.tile([128, 2048], F32, tag="psS")   # 4 banks

            for gi in range(SLOTS * repeat):
                g = gi % SLOTS
                nw = NW[g]
                W = 128 * nw
                cks = _chunks(W)

                # ---- MM1: causal scores row-tile (128, W), fp32; k-outer so
                # the stationary is reused across the chunk banks ----
                for k in range(8):
                    q = xt[:, k * T + W - 128:k * T + W]
                    for j0, n in cks:
                        nc.tensor.matmul(psS[:, j0:j0 + n], q,
                                         xt[:, k * T + j0:k * T + j0 + n],
                                         start=(k == 0), stop=(k == 7))

                # causal mask on the last 256 columns (in-place on PSUM)
                nc.vector.tensor_tensor(psS[:, W - 256:W], psS[:, W - 256:W],
                                        msk[:], ALU.min)

                # ---- top-8 values + indices (straight from PSUM) ----
                m8 = sm.tile([128, 8], F32, tag="m8")
                nc.vector.max(m8[:], psS[:, :W])
                ix8 = sm.tile([128, 8], U16, tag="ix8")
                nc.vector.max_index(ix8[:], m8[:], psS[:, :W])

                # ---- gather index tile: position n = r*128 + i lives at
                # idxs[n%16, n//16] = idxs[i%16, 8r + i//16]; build group 0
                # with 8 strided DMAs, replicate to the other 7 groups ----
                idxs = ixp.tile([128, 64], I16, tag="idxs")
                for ib in range(8):
                    nc.sync.dma_start(idxs[0:16, ib:64:8].bitcast(U16),
                                      ix8[16 * ib:16 * ib + 16, :])
                for kk in range(1, 8):
                    nc.sync.dma_start(idxs[16 * kk:16 * kk + 16, :],
                                      idxs[0:16, :])

                # ---- gather all 1024 neighbor rows from HBM in one go ----
                gath = gtp.tile([128, 8 * D], F16, tag="gath")
                nc.gpsimd.dma_gather(
                    gath[:].rearrange("p (r d) -> p r d", d=D), xg_d[:],
                    idxs[:], num_idxs=1024, num_idxs_reg=1024, elem_size=D)

                # ---- msg*deg: pairwise-tree sum of the 8 gathered rows ----
                s1 = gtp.tile([128, 4 * D], F32, tag="s1")
                nc.vector.tensor_tensor(s1[:], gath[:, :4 * D],
                                        gath[:, 4 * D:], ALU.add)
                s2 = gtp.tile([128, 2 * D], F32, tag="s2")
                nc.vector.tensor_tensor(s2[:], s1[:, :2 * D], s1[:, 2 * D:],
                                        ALU.add)
                msum = gtp.tile([128, D], F32, tag="msum")
                nc.vector.tensor_tensor(msum[:], s2[:, :D], s2[:, D:],
                                        ALU.add)

                # ---- blend (deg-divide via precomputed sv), fp16 out ----
                if g == 0:
                    blall = blp.tile([128, 8 * D], F16, tag="blall",
                                     name=f"blall{gi}")
                nc.vector.scalar_tensor_tensor(
                    blall[:, g * D:(g + 1) * D], msum[:], sv[:, g:g + 1],
                    xr[:, g * D:(g + 1) * D], op0=ALU.mult, op1=ALU.add)

                # ---- once per iteration: one wide Gelu + one output DMA ----
                if g == SLOTS - 1:
                    outsb = obp.tile([128, 8 * D], F16, tag="outsb")
                    nc.scalar.activation(outsb[:], blall[:], AF.Gelu)
                    nc.sync.dma_start(out_d[:], outsb[:])

    nc.finalize()
    return nc


def _prep_inputs(x, gain, bias, log_mix, log_scale):
    """Build the 8 per-core input maps."""
    x = np.asarray(x, dtype=np.float32)
    gain = np.asarray(gain, dtype=np.float32)
    bias = np.asarray(bias, dtype=np.float32)
    mix = np.float32(1.0) / (np.float32(1.0) + np.exp(-np.asarray(log_mix, np.float32)))
    scale = np.log1p(np.exp(np.asarray(log_scale, np.float32))).astype(np.float32) + np.float32(0.01)
    one_minus_mix = np.float32(1.0) - mix

    tril = np.tril(np.ones((128, 128), np.bool_))
    tril_bias = np.where(tril, BIG, -BIG).astype(np.float32)
    keep = np.full((128, 128), BIG, np.float32)
    kill = np.full((128, 128), -BIG, np.float32)

    in_maps = []
    meta = []
    for c in range(NCORES):
        b = c % 4
        grp = c // 4
        if grp == 0:
            perm_blocks = np.arange(16)
            tiles = [15 - 2 * g for g in range(SLOTS)]
            msk = np.concatenate([keep, tril_bias], axis=1)
        else:
            perm_blocks = np.arange(16).reshape(8, 2)[:, ::-1].ravel()
            tiles = [14 - 2 * g for g in range(SLOTS)]
            msk = np.concatenate([kill, tril_bias], axis=1)

        # sv[p, g] = (1-mix)/deg, deg = min(global_row+1, 8) is deterministic
        sv = np.empty((128, 8), np.float32)
        for g in range(SLOTS):
            rows = 128 * tiles[g] + np.arange(128)
            deg = np.minimum(rows + 1, 8).astype(np.float32)
            sv[:, g] = one_minus_mix / deg

        perm_rows = (perm_blocks[:, None] * 128 + np.arange(128)[None, :]).ravel()
        xp = x[b][perm_rows]  # (T, D) permuted rows
        # xt: (128, 8*T), chunk k = x^T[128k:128k+128, :]
        xt = np.ascontiguousarray(
            xp.T.reshape(8, 128, T).transpose(1, 0, 2).reshape(128, 8 * T))
        # xg: (T, D) fp16 (x*gain) permuted rows; dma_gather source
        xg = (xp * gain[None, :]).astype(np.float16)
        # xr: (128, 8*D) fp16 slot-major mix*gain*x + bias (true row order)
        xr = np.empty((128, 8 * D), np.float16)
        for g in range(SLOTS):
            r = 128 * tiles[g]
            xr[:, g * D:(g + 1) * D] = ((mix * gain[None, :]) * x[b, r:r + 128, :] + bias[None, :]).astype(np.float16)
        in_maps.append({
            "xt": xt, "xg": xg, "xr": xr, "msk": msk, "sv": sv,
        })
        meta.append((b, tiles, scale))
    return in_maps, meta


def _host_head_rows(x, gain, bias, mix, scale, nrows=7):
    """Exact outputs for rows 0..nrows-1 of each batch (deg < 8 there: ALL
    causal candidates are selected, so msg is the causal running mean)."""
    from scipy.special import erf
    xh = x[:, :nrows, :].astype(np.float64)  # (B, nrows, D)
    csum = np.cumsum(xh, axis=1)
    deg = np.arange(1, nrows + 1, dtype=np.float64)[None, :, None]
    msg = csum / deg
    blended = mix * xh + (1.0 - mix) * msg
    z = blended * gain[None, None, :].astype(np.float64) + bias[None, None, :]
    g = 0.5 * z * (1.0 + erf(z / np.sqrt(2.0)))
    return (g * scale).astype(np.float32)


def kernel(x, gain, bias, log_mix, log_scale):
    if "nc" not in _cache:
        _cache["nc"] = _build_program()
    nc = _cache["nc"]
    x = np.asarray(x, dtype=np.float32)
    gain = np.asarray(gain, dtype=np.float32)
    bias = np.asarray(bias, dtype=np.float32)
    in_maps, meta = _prep_inputs(x, gain, bias, log_mix, log_scale)
    res = run_bass_kernel_spmd(nc, in_maps, core_ids=list(range(NCORES)))
    y = np.empty((B, T, D), np.float32)
    for c in range(NCORES):
        b, tiles, scale = meta[c]
        o = res.results[c]["out"].astype(np.float32) * scale  # (128, 8*D)
        for g in range(SLOTS):
            r = 128 * tiles[g]
            y[b, r:r + 128, :] = o[:, g * D:(g + 1) * D]
    # rows 0..6 of each batch: fewer than 8 causal candidates; the device's
    # max8_index picks masked entries there, so compute those exactly here
    mix = np.float32(1.0) / (np.float32(1.0) + np.exp(-np.asarray(log_mix, np.float32)))
    scale = np.log1p(np.exp(np.asarray(log_scale, np.float32))).astype(np.float32) + np.float32(0.01)
    y[:, :7, :] = _host_head_rows(x, gain, bias, float(mix), float(scale))
    return y


# revision 11
# speedup vs baseline: 3.4979x; 1.2444x over previous
"""Trainium2 Bass kernel for causal top-K GNN message passing.

reference semantics (B=4, T=2048, D=1024, K=8):
    scores = x @ x^T per batch, causal (j <= i)
    A[i,j] = 1 iff j among top-8 causal scores of row i
    msg    = (A @ x) / deg
    out    = gelu(mix*x + (1-mix)*msg) * scale       (gain=*, bias=+ general)

Strategy (8 NeuronCores, SPMD single program):
  - core c handles batch b = c % 4; cores 0-3 take row-tiles t = 15-2g
    (slot g = 0..7), cores 4-7 take t = 14-2g.
  - slot g is compiled for causal width W_g = 128*(16-2g) columns; cores 4-7
    use a per-core pair-swapped row-block permutation of the key/value axis so
    their row-tile lands in the last 128 columns of the slot's width. All
    per-core variation lives in the host-prepared input data; the device
    program is identical across cores.
  - This backend executes instructions serially at a roughly flat per-
    instruction cost (matmul ~60-90us, DVE-f32 ~20-49us, ACT ~100-126us,
    small DMA ~15us, cross-engine sync ~50-100us), so the kernel minimizes
    weighted instruction count:
    * scores in ONE fp32 matmul per (k-chunk, 512-col chunk): 160 calls/iter,
      k-outer so the stationary is reused and accumulation chains interleave
      across PSUM banks; the top-8 DVE ops read the PSUM scores directly.
    * top-8 NEIGHBOR INDICES via max8 + max8_index (no thresholding, no 0/1
      adjacency matrix, no transposes, no second matmul): the 8 indices per
      query drive ONE gpsimd dma_gather that pulls all 1024 neighbor rows of
      x straight from HBM, grouped so each query's 8 rows land in its own
      partition. The gather index tile is built with 8 strided 2-byte DMAs
      (position n = r*128 + i lives at idxs[n%16, n//16]) and must be
      replicated into all 8 groups of 16 partitions (HW DGE cores each read
      their own group; CoreSim only reads group 0).
    * msg*deg = sum of the 8 gathered rows = 3 wide DVE adds (pairwise tree).
    * deg is deterministic (min(row+1, 8)), so (1-mix)/deg ships as a host
      precomputed per-partition constant; one 1024-wide scalar_tensor_tensor
      per slot blends msg with mix*gain*x+bias (fp16 out), then a single
      8192-wide Gelu and one output DMA per iteration.
    * rows 0-6 of each batch have fewer than 8 causal candidates; max8_index
      picks masked entries there, so the host overwrites those 28 rows with
      the exact (trivial: msg = causal running mean) fp32 computation. The
      final *scale is also applied on the host after gather.
"""

import sys
import types

try:
    import concourse  # provided by the runtime environment (axon site)
except ImportError:
    sys.path.insert(0, "/opt/trn_rl_repo")

# run_bass_kernel_spmd imports antenv.axon_hooks when BASS_TRACE is set; the
# module is absent in this image, so provide a no-trace stub.
try:
    import antenv.axon_hooks  # noqa: F401
except ImportError:
    _m = types.ModuleType("antenv.axon_hooks")
    _m.get_axon_ntff_profile_hook = lambda: None
    sys.modules["antenv.axon_hooks"] = _m

import numpy as np

import concourse.bacc as bacc
import concourse.tile as tile
import concourse.mybir as mybir
from concourse.bass_utils import run_bass_kernel_spmd

F32 = mybir.dt.float32
F16 = mybir.dt.float16
U16 = mybir.dt.uint16
I16 = mybir.dt.int16
AF = mybir.ActivationFunctionType
ALU = mybir.AluOpType
AX = mybir.AxisListType

B, T, D, K = 4, 2048, 1024, 8
NCORES = 8
SLOTS = 8
NW = [16 - 2 * g for g in range(SLOTS)]  # slot widths in 128-blocks
BIG = np.float32(3e38)

_cache = {}


def _chunks(w):
    """split [0, w) into <=512 pieces"""
    out = []
    j = 0
    while j < w:
        n = min(512, w - j)
        out.append((j, n))
        j += n
    return out


def _build_program(repeat=1):
    nc = bacc.Bacc("TRN2", target_bir_lowering=False, debug=False,
                   num_devices=NCORES)

    # ---- DRAM I/O (per-core shapes; SPMD identical program) ----
    # fp32 x^T, d-chunk major: [:, k*T + j] = x[b, perm(j), 128k+p]
    xt_d = nc.declare_dram_parameter("xt", [128, 8 * T], F32, isOutput=False)
    # fp16 (x*gain) in permuted row order; dma_gather source (stays in HBM)
    xg_d = nc.declare_dram_parameter("xg", [T, D], F16, isOutput=False)
    # mix*gain*x + bias rows, slot major, fp16 (true row order)
    xr_d = nc.declare_dram_parameter("xr", [128, 8 * D], F16, isOutput=False)
    # causal mask bias for the last 256 columns of each slot
    msk_d = nc.declare_dram_parameter("msk", [128, 256], F32, isOutput=False)
    # per-partition constants: col g = (1-mix)/deg(core, slot g, partition)
    sv_d = nc.declare_dram_parameter("sv", [128, 8], F32, isOutput=False)
    out_d = nc.declare_dram_parameter("out", [128, 8 * D], F16, isOutput=True)

    with tile.TileContext(nc) as tc:
        with (
            tc.tile_pool(name="cst", bufs=1) as cst,
            tc.tile_pool(name="sm", bufs=1) as sm,
            tc.tile_pool(name="ixp", bufs=1) as ixp,
            tc.tile_pool(name="gt", bufs=1) as gtp,
            tc.tile_pool(name="bl", bufs=1) as blp,
            tc.tile_pool(name="ob", bufs=1) as obp,
            tc.tile_pool(name="psS", bufs=1, space="PSUM") as psS_p,
        ):
            xt = cst.tile([128, 8 * T], F32, tag="xt")
            xr = cst.tile([128, 8 * D], F16, tag="xr")
            msk = cst.tile([128, 256], F32, tag="msk")
            sv = cst.tile([128, 8], F32, tag="sv")
            nc.sync.dma_start(xt[:], xt_d[:])
            nc.sync.dma_start(xr[:], xr_d[:])
            nc.sync.dma_start(msk[:], msk_d[:])
            nc.sync.dma_start(sv[:], sv_d[:])

            psS = psS_p.tile([128, 2048], F32, tag="psS")   # 4 banks

            for gi in range(SLOTS * repeat):
                g = gi % SLOTS
                nw = NW[g]
                W = 128 * nw
                cks = _chunks(W)

                # ---- MM1: causal scores row-tile (128, W), fp32; k-outer so
                # the stationary is reused across the chunk banks ----
                for k in range(8):
                    q = xt[:, k * T + W - 128:k * T + W]
                    for j0, n in cks:
                        nc.tensor.matmul(psS[:, j0:j0 + n], q,
                                         xt[:, k * T + j0:k * T + j0 + n],
                                         start=(k == 0), stop=(k == 7))

                # causal mask on the last 256 columns (in-place on PSUM)
                nc.vector.tensor_tensor(psS[:, W - 256:W], psS[:, W - 256:W],
                                        msk[:], ALU.min)

                # ---- top-8 values + indices (straight from PSUM) ----
                m8 = sm.tile([128, 8], F32, tag="m8")
                nc.vector.max(m8[:], psS[:, :W])
                ix8 = sm.tile([128, 8], U16, tag="ix8")
                nc.vector.max_index(ix8[:], m8[:], psS[:, :W])

                # ---- gather index tile: position n = r*128 + i lives at
                # idxs[n%16, n//16] = idxs[i%16, 8r + i//16]; build group 0
                # with 8 strided DMAs, replicate to the other 7 groups ----
                idxs = ixp.tile([128, 64], I16, tag="idxs")
                for ib in range(8):
                    nc.sync.dma_start(idxs[0:16, ib:64:8].bitcast(U16),
                                      ix8[16 * ib:16 * ib + 16, :])
                for sz in (16, 32, 64):  # doubling replication: 3 DMAs
                    nc.sync.dma_start(idxs[sz:2 * sz, :], idxs[0:sz, :])

                # ---- gather all 1024 neighbor rows from HBM in one go ----
                gath = gtp.tile([128, 8 * D], F16, tag="gath")
                nc.gpsimd.dma_gather(
                    gath[:].rearrange("p (r d) -> p r d", d=D), xg_d[:],
                    idxs[:], num_idxs=1024, num_idxs_reg=1024, elem_size=D)

                # ---- msg*deg: pairwise-tree sum of the 8 gathered rows ----
                s1 = gtp.tile([128, 4 * D], F32, tag="s1")
                nc.vector.tensor_tensor(s1[:], gath[:, :4 * D],
                                        gath[:, 4 * D:], ALU.add)
                s2 = gtp.tile([128, 2 * D], F32, tag="s2")
                nc.vector.tensor_tensor(s2[:], s1[:, :2 * D], s1[:, 2 * D:],
                                        ALU.add)
                msum = gtp.tile([128, D], F32, tag="msum")
                nc.vector.tensor_tensor(msum[:], s2[:, :D], s2[:, D:],
                                        ALU.add)

                # ---- blend (deg-divide via precomputed sv), fp16 out ----
                if g == 0:
                    blall = blp.tile([128, 8 * D], F16, tag="blall",
                                     name=f"blall{gi}")
                nc.vector.scalar_tensor_tensor(
                    blall[:, g * D:(g + 1) * D], msum[:], sv[:, g:g + 1],
                    xr[:, g * D:(g + 1) * D], op0=ALU.mult, op1=ALU.add)

                # ---- once per iteration: one wide Gelu + one output DMA ----
                if g == SLOTS - 1:
                    outsb = obp.tile([128, 8 * D], F16, tag="outsb")
                    nc.scalar.activation(outsb[:], blall[:], AF.Gelu)
                    nc.sync.dma_start(out_d[:], outsb[:])

    nc.finalize()
    return nc


def _prep_inputs(x, gain, bias, log_mix, log_scale):
    """Build the 8 per-core input maps."""
    x = np.asarray(x, dtype=np.float32)
    gain = np.asarray(gain, dtype=np.float32)
    bias = np.asarray(bias, dtype=np.float32)
    mix = np.float32(1.0) / (np.float32(1.0) + np.exp(-np.asarray(log_mix, np.float32)))
    scale = np.log1p(np.exp(np.asarray(log_scale, np.float32))).astype(np.float32) + np.float32(0.01)
    one_minus_mix = np.float32(1.0) - mix

    tril = np.tril(np.ones((128, 128), np.bool_))
    tril_bias = np.where(tril, BIG, -BIG).astype(np.float32)
    keep = np.full((128, 128), BIG, np.float32)
    kill = np.full((128, 128), -BIG, np.float32)

    in_maps = []
    meta = []
    for c in range(NCORES):
        b = c % 4
        grp = c // 4
        if grp == 0:
            perm_blocks = np.arange(16)
            tiles = [15 - 2 * g for g in range(SLOTS)]
            msk = np.concatenate([keep, tril_bias], axis=1)
        else:
            perm_blocks = np.arange(16).reshape(8, 2)[:, ::-1].ravel()
            tiles = [14 - 2 * g for g in range(SLOTS)]
            msk = np.concatenate([kill, tril_bias], axis=1)

        # sv[p, g] = (1-mix)/deg, deg = min(global_row+1, 8) is deterministic
        sv = np.empty((128, 8), np.float32)
        for g in range(SLOTS):
            rows = 128 * tiles[g] + np.arange(128)
            deg = np.minimum(rows + 1, 8).astype(np.float32)
            sv[:, g] = one_minus_mix / deg

        perm_rows = (perm_blocks[:, None] * 128 + np.arange(128)[None, :]).ravel()
        xp = x[b][perm_rows]  # (T, D) permuted rows
        # xt: (128, 8*T), chunk k = x^T[128k:128k+128, :]
        xt = np.ascontiguousarray(
            xp.T.reshape(8, 128, T).transpose(1, 0, 2).reshape(128, 8 * T))
        # xg: (T, D) fp16 (x*gain) permuted rows; dma_gather source
        xg = (xp * gain[None, :]).astype(np.float16)
        # xr: (128, 8*D) fp16 slot-major mix*gain*x + bias (true row order)
        xr = np.empty((128, 8 * D), np.float16)
        for g in range(SLOTS):
            r = 128 * tiles[g]
            xr[:, g * D:(g + 1) * D] = ((mix * gain[None, :]) * x[b, r:r + 128, :] + bias[None, :]).astype(np.float16)
        in_maps.append({
            "xt": xt, "xg": xg, "xr": xr, "msk": msk, "sv": sv,
        })
        meta.append((b, tiles, scale))
    return in_maps, meta


def _host_head_rows(x, gain, bias, mix, scale, nrows=7):
    """Exact outputs for rows 0..nrows-1 of each batch (deg < 8 there: ALL
    causal candidates are selected, so msg is the causal running mean)."""
    import math
    erf = np.vectorize(math.erf)
    xh = x[:, :nrows, :].astype(np.float64)  # (B, nrows, D)
    csum = np.cumsum(xh, axis=1)
    deg = np.arange(1, nrows + 1, dtype=np.float64)[None, :, None]
    msg = csum / deg
    blended = mix * xh + (1.0 - mix) * msg
    z = blended * gain[None, None, :].astype(np.float64) + bias[None, None, :]
    g = 0.5 * z * (1.0 + erf(z / np.sqrt(2.0)))
    return (g * scale).astype(np.float32)


def kernel(x, gain, bias, log_mix, log_scale):
    if "nc" not in _cache:
        _cache["nc"] = _build_program()
    nc = _cache["nc"]
    x = np.asarray(x, dtype=np.float32)
    gain = np.asarray(gain, dtype=np.float32)
    bias = np.asarray(bias, dtype=np.float32)
    in_maps, meta = _prep_inputs(x, gain, bias, log_mix, log_scale)
    res = run_bass_kernel_spmd(nc, in_maps, core_ids=list(range(NCORES)))
    y = np.empty((B, T, D), np.float32)
    for c in range(NCORES):
        b, tiles, scale = meta[c]
        o = res.results[c]["out"].astype(np.float32) * scale  # (128, 8*D)
        for g in range(SLOTS):
            r = 128 * tiles[g]
            y[b, r:r + 128, :] = o[:, g * D:(g + 1) * D]
    # rows 0..6 of each batch: fewer than 8 causal candidates; the device's
    # max8_index picks masked entries there, so compute those exactly here
    mix = np.float32(1.0) / (np.float32(1.0) + np.exp(-np.asarray(log_mix, np.float32)))
    scale = np.log1p(np.exp(np.asarray(log_scale, np.float32))).astype(np.float32) + np.float32(0.01)
    y[:, :7, :] = _host_head_rows(x, gain, bias, float(mix), float(scale))
    return y
